# revision 33
# baseline (speedup 1.0000x reference)
"""Trainium2 Bass kernel for a continuous bilinear Koopman operator rollout.

Problem (hardcoded shapes): z0 [256, 256] f32, kernel [256, 256] f32,
log_dt scalar, T=512.  Output: [256, 512, 256] f32 with
out[:, t, :] = z0 @ K_discrete^(t+1),
K_discrete = (I - 0.5*dt*K)^-1 (I + 0.5*dt*K), dt = exp(log_dt).

Strategy (v5):
  - Host (f64) computes K_discrete, powers A^1..A^16, and the 32
    chunk-start states s_k = z0 @ A^(16k).  z0/output sharded across 8
    cores along batch (32 trajectories per core).
  - Device computes out-chunk rows s_k @ A^j (j=1..16) as matmuls:
    per group (4 chunks x 32 batch = 128 partitions), PSUM tiles
    [128, 1024] accumulate over the contraction r=256.
  - Chunks 0..23 (~12% of output energy) are computed entirely in fp8
    (e4m3) with DoubleRow perf mode: K=256 in one matmul at 2x PE
    throughput; chunks 24..31 in bf16.
  - Chunks 0..23 are written to HBM as scaled fp8 (1 B/elem); chunks
    24..31 as bf16.  Host decodes.  Predicted rel-err ~1.18e-2 vs the
    2e-2 gate (validated with an exact host-side quantization sim).
  - Input DMAs are split in consumption order across both HWDGE
    queues so the first matmul starts right after the preamble; dummy
    matmuls on a scratch tile pre-warm the PE HAM clock gate during
    the input-DMA window.
  - PSUM->SBUF cast copies (the throughput wall, PSUM reads are 1
    elem/lane/cycle) are balanced DVE(17)/ACT(15) since ACT also
    issues DMA descriptors.
  - Output drains are per-group-half 128-partition DMAs with 8KB/4KB
    contiguous per-partition descriptors alternating across the two
    HWDGE queues; the final group's drain is split across both queues
    to halve the tail.
"""

import numpy as np

B = 256
D = 256
T = 512
N_CORES = 8
B_LOC = B // N_CORES      # 32
C = 16                    # chunk length (powers A^1..A^C shipped)
N_CHUNKS = T // C         # 32
N_GROUPS = N_CHUNKS // 4  # 8 groups of 4 chunks -> M=128

# group processing order: bf16-computed groups first (their operands
# arrive first), fp8 groups after (smaller drain tail).
GORDER = [6, 7, 5, 0, 1, 2, 3, 4]
G16C = [6, 7]                 # bf16-computed groups (chunks 24..31)
G8C = [0, 1, 2, 3, 4, 5]      # fp8-computed groups (chunks 0..23)
FP8W = {0, 1, 2, 3, 4, 5}     # fp8-written groups (chunks 0..23)
SIDX = {6: 0, 7: 1}           # S16 block index per bf16 group

# qin16 [128, 8704] bf16:
#   [0:512)    S16 blocks, one 256-col block per gi in (6,7):
#              col gi*256 + h*128 + (a*32+b) = s_{4g+a}[cb+b, h*128+r]
#   [512:8704) P16: col 512 + pp*2048 + q*1024 + h*512 + u2*256 + d
#              = A^{4pp+1+2q+u2}[h*128+r, d]
S16_COLS = len(G16C) * 256          # 512
P16_BASE = S16_COLS
Q16_COLS = S16_COLS + 8192          # 8704
# qin8 [128, 9728] fp8, viewed [128, 2(h), 4864]:
#   within h: [0:768) S8 blocks per g in G8C (scaled by s8sc);
#             [768:4864) P8 same (pp,q,u2,d) order as P16 (scaled p8sc)
Q8_S = len(G8C) * 128               # 768
Q8_H = Q8_S + 4096                  # 4864

_CACHE = {}


def _build_bass(eff8, f8s, p8sc):
    import concourse.tile as tile
    from concourse import bacc, mybir

    f32 = mybir.dt.float32
    bf16 = mybir.dt.bfloat16
    fp8 = mybir.dt.float8e4
    DR = mybir.MatmulPerfMode.DoubleRow
    nc = bacc.Bacc("TRN2", target_bir_lowering=False, debug=False)

    qin16 = nc.dram_tensor("qin16", [128, Q16_COLS], bf16, kind="ExternalInput").ap()
    qin8 = nc.dram_tensor("qin8", [128, 2 * Q8_H], fp8, kind="ExternalInput").ap()
    out16 = nc.dram_tensor("out16", [128, 8192], bf16, kind="ExternalOutput").ap()
    out8 = nc.dram_tensor("out8", [128, 24576], fp8, kind="ExternalOutput").ap()
    qin8v = qin8.rearrange("p (h c) -> p h c", h=2)

    with tile.TileContext(nc) as tc:
        with (
            tc.tile_pool(name="const", bufs=1) as cpool,
            tc.tile_pool(name="psum", bufs=4, space="PSUM") as psum_pool,
        ):
            Q16 = cpool.tile([128, Q16_COLS], bf16, name="q16")
            Q8 = cpool.tile([128, 2, Q8_H], fp8, name="q8")
            st16 = cpool.tile([128, 8192], bf16, name="st16")
            st8a = cpool.tile([128, 8192], fp8, name="st8a")
            st8b = cpool.tile([128, 8192], fp8, name="st8b")
            st8c = cpool.tile([128, 8192], fp8, name="st8c")
            scratch = cpool.tile([128, 512], bf16, name="scratch")
            tbl = cpool.tile([128, 8], bf16, name="tbl")
            sthalf = {6: (st16, 0), 7: (st16, 1), 5: (st8a, 0), 0: (st8a, 1),
                      1: (st8b, 0), 2: (st8b, 1), 3: (st8c, 0), 4: (st8c, 1)}

            nc.gpsimd.memset(scratch[:], 0.0)

            # ---- input DMAs in consumption order.  SDMA engines drain
            # BOTH queues round-robin at packet granularity, so arrival
            # is global-byte-order; fine (q-half) granularity makes each
            # completion semaphore fire as early as possible. ----
            nc.sync.dma_start(Q16[:, 0:512], qin16[:, 0:512])            # S16
            for pp in range(4):
                b = P16_BASE + pp * 2048
                nc.scalar.dma_start(Q16[:, b:b + 1024], qin16[:, b:b + 1024])
                nc.sync.dma_start(Q16[:, b + 1024:b + 2048],
                                  qin16[:, b + 1024:b + 2048])
            nc.scalar.dma_start(Q8[:, 0:1, 0:2816], qin8v[:, 0:1, 0:2816])
            nc.sync.dma_start(Q8[:, 1:2, 0:2816], qin8v[:, 1:2, 0:2816])
            nc.scalar.dma_start(Q8[:, 0:1, 2816:Q8_H], qin8v[:, 0:1, 2816:Q8_H])
            nc.sync.dma_start(Q8[:, 1:2, 2816:Q8_H], qin8v[:, 1:2, 2816:Q8_H])
            # ACT table preload (after scalar's DMA issues, before copies)
            nc.scalar.copy(tbl[:, 0:8], scratch[:, 0:8])

            # PE warm-up: dummy matmuls on scratch so HAM reaches K=8/8
            # while the first input DMAs are still in flight.
            ps_pre = psum_pool.tile([128, 1024], f32, name="ps", tag="ps")
            for _ in range(6):
                nc.tensor.matmul(ps_pre[:, 0:512], scratch[:, 0:128],
                                 scratch[:, 0:512], start=True, stop=True)

            # 32 PSUM->SBUF cast copies, balanced DVE(17)/ACT(15) via
            # error diffusion so the split stays even over time.
            state = {"ct": 0, "acc": 0.0}

            def emit_copy(ps, g, pp):
                stt, half = sthalf[g]
                dst = stt[:, half * 4096 + pp * 1024: half * 4096 + (pp + 1) * 1024]
                state["acc"] += 15.0 / 32.0
                on_act = state["acc"] >= 1.0
                if on_act:
                    state["acc"] -= 1.0
                if g in FP8W:
                    sc = eff8 if g in G8C else f8s
                    if on_act:
                        nc.scalar.mul(dst, ps[:], sc)
                    else:
                        nc.vector.tensor_scalar_mul(dst, ps[:], sc)
                else:
                    if on_act:
                        nc.scalar.copy(dst, ps[:])
                    else:
                        nc.vector.tensor_copy(dst, ps[:])

            def mm16(ps, g, pp, q):
                sb = SIDX[g] * 256
                for h in range(2):
                    pb = P16_BASE + pp * 2048 + q * 1024 + h * 512
                    nc.tensor.matmul(
                        ps[:, q * 512:(q + 1) * 512],
                        Q16[:, sb + h * 128: sb + (h + 1) * 128],
                        Q16[:, pb:pb + 512],
                        start=(h == 0), stop=(h == 1),
                    )

            # bf16 phase: groups 6 and 7 interleaved at (pp, q) level so
            # consumption (~295 GB/s) matches input arrival (~410 GB/s)
            # with no PE stall long enough to re-throttle the HAM gate.
            for pp in range(4):
                if pp == 0:
                    t6 = ps_pre
                else:
                    t6 = psum_pool.tile([128, 1024], f32, name="ps", tag="ps")
                t7 = psum_pool.tile([128, 1024], f32, name="ps", tag="ps")
                for q in range(2):
                    mm16(t6, 6, pp, q)
                    mm16(t7, 7, pp, q)
                emit_copy(t6, 6, pp)
                emit_copy(t7, 7, pp)
            nc.sync.dma_start(out16[:, 0:4096], st16[:, 0:4096])
            nc.scalar.dma_start(out16[:, 4096:8192], st16[:, 4096:8192])

            # fp8 phase: sequential groups
            for g in (5, 0, 1, 2, 3, 4):
                for pp in range(4):
                    ps = psum_pool.tile([128, 1024], f32, name="ps", tag="ps")
                    si = G8C.index(g)
                    for q in range(2):
                        pb = Q8_S + pp * 1024 + q * 512
                        nc.tensor.matmul(
                            ps[:, q * 512:(q + 1) * 512],
                            Q8[:, :, si * 128:(si + 1) * 128],
                            Q8[:, :, pb:pb + 512],
                            start=True, stop=True, perf_mode=DR,
                        )
                    emit_copy(ps, g, pp)
                # drains: per-half 128-partition DMAs (4KB fp8 descriptors)
                # from the idle sync engine so scalar stays free for
                # copies; the final drain splits across both queues.
                stt, half = sthalf[g]
                if g == 4:
                    nc.sync.dma_start(out8[:, 20480:22528], st8c[:, 4096:6144])
                    nc.scalar.dma_start(out8[:, 22528:24576], st8c[:, 6144:8192])
                else:
                    dcol = {5: 0, 0: 4096, 1: 8192, 2: 12288, 3: 16384}[g]
                    nc.sync.dma_start(out8[:, dcol:dcol + 4096],
                                      stt[:, half * 4096:(half + 1) * 4096])

    nc.compile()
    return nc


def _pow2floor(x):
    return float(2.0 ** np.floor(np.log2(x)))


def _host_prep(z0, kernel, log_dt):
    """fp64 host math: K_discrete, powers, chunk starts; pack qin16/qin8."""
    import ml_dtypes

    BF16 = ml_dtypes.bfloat16
    FP8NP = ml_dtypes.float8_e4m3

    K = np.asarray(kernel, dtype=np.float64)
    dt = float(np.exp(np.float64(np.asarray(log_dt))))
    eye = np.eye(D, dtype=np.float64)
    A = np.linalg.solve(eye - 0.5 * dt * K, eye + 0.5 * dt * K)

    pows = [None] * (C + 1)
    pows[1] = A
    for j in range(2, C + 1):
        pows[j] = pows[j - 1] @ A

    z = np.asarray(z0, dtype=np.float64)
    s_list = [z]
    for _ in range(N_CHUNKS - 1):
        s_list.append(s_list[-1] @ pows[C])
    s_all = np.stack(s_list, axis=0)  # [32, B, D]

    # scales (powers of two)
    s8max = max(float(np.abs(s_all[4 * g: 4 * g + 4]).max()) for g in G8C)
    s8sc = _pow2floor(240.0 / (1.05 * s8max))
    p8max = max(float(np.abs(pows[j]).max()) for j in range(1, C + 1))
    p8sc = _pow2floor(240.0 / (1.05 * p8max))
    n_fp8w_chunks = 4 * len(FP8W)
    rownorm = max(
        float(np.linalg.norm(s_all[k], axis=1).max()) for k in range(n_fp8w_chunks)
    )
    colnorm = max(
        float(np.linalg.norm(pows[j], axis=0).max()) for j in range(1, C + 1)
    )
    f8s = _pow2floor(240.0 / (1.05 * rownorm * colnorm))
    eff8 = f8s / (s8sc * p8sc)

    # P16 region [128, 8192]
    p16 = np.empty((128, 8192), dtype=np.float64)
    for pp in range(4):
        for q in range(2):
            for h in range(2):
                for u2 in range(2):
                    j = 4 * pp + 1 + 2 * q + u2
                    base = pp * 2048 + q * 1024 + h * 512 + u2 * 256
                    p16[:, base:base + 256] = pows[j][h * 128:(h + 1) * 128, :]

    # P8 per-h region [2][128, 4096]
    p8h = np.empty((2, 128, 4096), dtype=np.float64)
    for h in range(2):
        for pp in range(4):
            for q in range(2):
                for u2 in range(2):
                    j = 4 * pp + 1 + 2 * q + u2
                    col = pp * 1024 + q * 512 + u2 * 256
                    p8h[h][:, col:col + 256] = (
                        pows[j][h * 128:(h + 1) * 128, :] * p8sc
                    )

    def s_block(g, cb, h):
        # [128 r, 128 (a*32+b)] = s_{4g+a}[cb+b, h*128+r]
        blk = s_all[4 * g: 4 * g + 4, cb:cb + 32, h * 128:(h + 1) * 128]
        return blk.transpose(2, 0, 1).reshape(128, 128)

    in_maps = []
    for c in range(N_CORES):
        cb = c * B_LOC
        q16 = np.empty((128, Q16_COLS), dtype=np.float64)
        for g in G16C:
            gi = SIDX[g]
            for h in range(2):
                q16[:, gi * 256 + h * 128: gi * 256 + (h + 1) * 128] = s_block(g, cb, h)
        q16[:, P16_BASE:] = p16
        q8 = np.empty((128, 2 * Q8_H), dtype=np.float64)
        for h in range(2):
            off = h * Q8_H
            for si, g in enumerate(G8C):
                q8[:, off + si * 128: off + (si + 1) * 128] = s_block(g, cb, h) * s8sc
            q8[:, off + Q8_S: off + Q8_H] = p8h[h]
        in_maps.append({
            "qin16": np.ascontiguousarray(q16).astype(BF16),
            "qin8": np.ascontiguousarray(q8).astype(FP8NP),
        })
    scales = {"fp8_scale": f8s, "eff8": eff8, "s8sc": s8sc, "p8sc": p8sc}
    return in_maps, scales


def _decode(res_c, f8s):
    """Device outputs [out16 [128,8192] bf16, out8 [128,24576] fp8]
    -> [B_LOC, T, D] f32."""
    o16 = np.asarray(res_c["out16"]).astype(np.float32)
    o8 = np.asarray(res_c["out8"]).astype(np.float32) / f8s
    out = np.empty((B_LOC, T, D), dtype=np.float32)

    def put(arr, g):  # arr [128, 4096]: [a*32+b, t_local*256+d]
        out[:, 4 * g * C:(4 * g + 4) * C, :] = (
            arr.reshape(4, 32, C, D).transpose(1, 0, 2, 3).reshape(32, 4 * C, D)
        )

    put(o16[:, 0:4096], 6)
    put(o16[:, 4096:8192], 7)
    for ri, (ga, gb) in enumerate([(5, 0), (1, 2), (3, 4)]):
        put(o8[:, ri * 8192: ri * 8192 + 4096], ga)
        put(o8[:, ri * 8192 + 4096: (ri + 1) * 8192], gb)
    return out


def kernel(**inputs):
    from concourse.bass_utils import run_bass_kernel_spmd

    z0 = inputs["z0"]
    kmat = inputs["kernel"]
    log_dt = inputs["log_dt"]
    t_in = int(np.asarray(inputs["T"]))
    assert t_in == T, f"kernel hardcoded for T={T}, got {t_in}"
    assert tuple(np.shape(z0)) == (B, D)

    in_maps, scales = _host_prep(z0, kmat, log_dt)

    key = (scales["fp8_scale"], scales["eff8"], scales["p8sc"])
    if _CACHE.get("key") != key:
        _CACHE["nc"] = _build_bass(scales["eff8"], scales["fp8_scale"],
                                   scales["p8sc"])
        _CACHE["key"] = key
    nc = _CACHE["nc"]

    res = run_bass_kernel_spmd(nc, in_maps, core_ids=list(range(N_CORES)))
    outs = [_decode(res.results[c], scales["fp8_scale"]) for c in range(N_CORES)]
    return np.concatenate(outs, axis=0)


# revision 34
# speedup vs baseline: 1.1429x; 1.1429x over previous
"""Trainium2 Bass kernel for a continuous bilinear Koopman operator rollout.

Problem (hardcoded shapes): z0 [256, 256] f32, kernel [256, 256] f32,
log_dt scalar, T=512.  Output: [256, 512, 256] f32 with
out[:, t, :] = z0 @ K_discrete^(t+1),
K_discrete = (I - 0.5*dt*K)^-1 (I + 0.5*dt*K), dt = exp(log_dt).

Strategy (v5):
  - Host (f64) computes K_discrete, powers A^1..A^16, and the 32
    chunk-start states s_k = z0 @ A^(16k).  z0/output sharded across 8
    cores along batch (32 trajectories per core).
  - Device computes out-chunk rows s_k @ A^j (j=1..16) as matmuls:
    per group (4 chunks x 32 batch = 128 partitions), PSUM tiles
    [128, 1024] accumulate over the contraction r=256.
  - Chunks 0..23 (~12% of output energy) are computed entirely in fp8
    (e4m3) with DoubleRow perf mode: K=256 in one matmul at 2x PE
    throughput; chunks 24..31 in bf16.
  - Chunks 0..23 are written to HBM as scaled fp8 (1 B/elem); chunks
    24..31 as bf16.  Host decodes.  Predicted rel-err ~1.18e-2 vs the
    2e-2 gate (validated with an exact host-side quantization sim).
  - Input DMAs are split in consumption order across both HWDGE
    queues so the first matmul starts right after the preamble; dummy
    matmuls on a scratch tile pre-warm the PE HAM clock gate during
    the input-DMA window.
  - PSUM->SBUF cast copies (the throughput wall, PSUM reads are 1
    elem/lane/cycle) are balanced DVE(17)/ACT(15) since ACT also
    issues DMA descriptors.
  - Output drains are per-group-half 128-partition DMAs with 8KB/4KB
    contiguous per-partition descriptors alternating across the two
    HWDGE queues; the final group's drain is split across both queues
    to halve the tail.
"""

import numpy as np

B = 256
D = 256
T = 512
N_CORES = 8
B_LOC = B // N_CORES      # 32
C = 16                    # chunk length (powers A^1..A^C shipped)
N_CHUNKS = T // C         # 32
N_GROUPS = N_CHUNKS // 4  # 8 groups of 4 chunks -> M=128

# group processing order: bf16-computed groups first (their operands
# arrive first), fp8 groups after (smaller drain tail).
GORDER = [6, 7, 5, 0, 1, 2, 3, 4]
G16C = [6, 7]                 # bf16-computed groups (chunks 24..31)
G8C = [0, 1, 2, 3, 4, 5]      # fp8-computed groups (chunks 0..23)
FP8W = {0, 1, 2, 3, 4, 5}     # fp8-written groups (chunks 0..23)
SIDX = {6: 0, 7: 1}           # S16 block index per bf16 group

# qin16 [128, 8704] bf16:
#   [0:512)    S16 blocks, one 256-col block per gi in (6,7):
#              col gi*256 + h*128 + (a*32+b) = s_{4g+a}[cb+b, h*128+r]
#   [512:8704) P16: col 512 + pp*2048 + q*1024 + h*512 + u2*256 + d
#              = A^{4pp+1+2q+u2}[h*128+r, d]
S16_COLS = len(G16C) * 256          # 512
P16_BASE = S16_COLS
Q16_COLS = S16_COLS + 8192          # 8704
# qin8 [128, 9728] fp8, viewed [128, 2(h), 4864]:
#   within h: [0:768) S8 blocks per g in G8C (scaled by s8sc);
#             [768:4864) P8 same (pp,q,u2,d) order as P16 (scaled p8sc)
Q8_S = len(G8C) * 128               # 768
Q8_H = Q8_S + 4096                  # 4864

_CACHE = {}


def _build_bass(eff8, f8s, p8sc):
    import concourse.tile as tile
    from concourse import bacc, mybir

    f32 = mybir.dt.float32
    bf16 = mybir.dt.bfloat16
    fp8 = mybir.dt.float8e4
    DR = mybir.MatmulPerfMode.DoubleRow
    nc = bacc.Bacc("TRN2", target_bir_lowering=False, debug=False)

    qin16 = nc.dram_tensor("qin16", [128, Q16_COLS], bf16, kind="ExternalInput").ap()
    qin8 = nc.dram_tensor("qin8", [128, 2 * Q8_H], fp8, kind="ExternalInput").ap()
    out16 = nc.dram_tensor("out16", [128, 8192], bf16, kind="ExternalOutput").ap()
    out8 = nc.dram_tensor("out8", [128, 24576], fp8, kind="ExternalOutput").ap()
    qin8v = qin8.rearrange("p (h c) -> p h c", h=2)

    with tile.TileContext(nc) as tc:
        with (
            tc.tile_pool(name="const", bufs=1) as cpool,
            tc.tile_pool(name="psum", bufs=4, space="PSUM") as psum_pool,
        ):
            Q16 = cpool.tile([128, Q16_COLS], bf16, name="q16")
            Q8 = cpool.tile([128, 2, Q8_H], fp8, name="q8")
            st16 = cpool.tile([128, 8192], bf16, name="st16")
            st8a = cpool.tile([128, 8192], fp8, name="st8a")
            st8b = cpool.tile([128, 8192], fp8, name="st8b")
            st8c = cpool.tile([128, 8192], fp8, name="st8c")
            scratch = cpool.tile([128, 512], bf16, name="scratch")
            tbl = cpool.tile([128, 8], bf16, name="tbl")
            sthalf = {6: (st16, 0), 7: (st16, 1), 5: (st8a, 0), 0: (st8a, 1),
                      1: (st8b, 0), 2: (st8b, 1), 3: (st8c, 0), 4: (st8c, 1)}

            nc.gpsimd.memset(scratch[:], 0.0)

            # ---- input DMAs in consumption order.  SDMA engines drain
            # BOTH queues round-robin at packet granularity, so arrival
            # is global-byte-order; fine (q-half) granularity makes each
            # completion semaphore fire as early as possible. ----
            nc.sync.dma_start(Q16[:, 0:512], qin16[:, 0:512])            # S16
            for pp in range(4):
                b = P16_BASE + pp * 2048
                nc.scalar.dma_start(Q16[:, b:b + 1024], qin16[:, b:b + 1024])
                nc.sync.dma_start(Q16[:, b + 1024:b + 2048],
                                  qin16[:, b + 1024:b + 2048])
            nc.scalar.dma_start(Q8[:, 0:1, 0:2816], qin8v[:, 0:1, 0:2816])
            nc.sync.dma_start(Q8[:, 1:2, 0:2816], qin8v[:, 1:2, 0:2816])
            nc.scalar.dma_start(Q8[:, 0:1, 2816:Q8_H], qin8v[:, 0:1, 2816:Q8_H])
            nc.sync.dma_start(Q8[:, 1:2, 2816:Q8_H], qin8v[:, 1:2, 2816:Q8_H])
            # ACT table preload (after scalar's DMA issues, before copies)
            nc.scalar.copy(tbl[:, 0:8], scratch[:, 0:8])

            # PE warm-up: dummy matmuls on scratch so HAM reaches K=8/8
            # while the first input DMAs are still in flight.
            ps_pre = psum_pool.tile([128, 1024], f32, name="ps", tag="ps")
            for _ in range(6):
                nc.tensor.matmul(ps_pre[:, 0:512], scratch[:, 0:128],
                                 scratch[:, 0:512], start=True, stop=True)

            # 32 PSUM->SBUF cast copies, balanced DVE(17)/ACT(15) via
            # error diffusion so the split stays even over time.
            state = {"ct": 0, "acc": 0.0}

            def emit_copy(ps, g, pp):
                stt, half = sthalf[g]
                dst = stt[:, half * 4096 + pp * 1024: half * 4096 + (pp + 1) * 1024]
                state["acc"] += 15.0 / 32.0
                on_act = state["acc"] >= 1.0
                if on_act:
                    state["acc"] -= 1.0
                if g in FP8W:
                    sc = eff8 if g in G8C else f8s
                    if on_act:
                        nc.scalar.mul(dst, ps[:], sc)
                    else:
                        nc.vector.tensor_scalar_mul(dst, ps[:], sc)
                else:
                    if on_act:
                        nc.scalar.copy(dst, ps[:])
                    else:
                        nc.vector.tensor_copy(dst, ps[:])

            def mm16(ps, g, pp, q):
                sb = SIDX[g] * 256
                for h in range(2):
                    pb = P16_BASE + pp * 2048 + q * 1024 + h * 512
                    nc.tensor.matmul(
                        ps[:, q * 512:(q + 1) * 512],
                        Q16[:, sb + h * 128: sb + (h + 1) * 128],
                        Q16[:, pb:pb + 512],
                        start=(h == 0), stop=(h == 1),
                    )

            # bf16 phase: groups 6 and 7 interleaved at (pp, q) level so
            # consumption (~295 GB/s) matches input arrival (~410 GB/s)
            # with no PE stall long enough to re-throttle the HAM gate.
            for pp in range(4):
                if pp == 0:
                    t6 = ps_pre
                else:
                    t6 = psum_pool.tile([128, 1024], f32, name="ps", tag="ps")
                t7 = psum_pool.tile([128, 1024], f32, name="ps", tag="ps")
                for q in range(2):
                    mm16(t6, 6, pp, q)
                    mm16(t7, 7, pp, q)
                emit_copy(t6, 6, pp)
                emit_copy(t7, 7, pp)
            nc.sync.dma_start(out16[:, 0:4096], st16[:, 0:4096])
            nc.scalar.dma_start(out16[:, 4096:8192], st16[:, 4096:8192])

            # fp8 phase: sequential groups
            sched = [(g, pp) for g in (5, 0, 1, 2, 3, 4) for pp in range(4)]
            for g, pp in sched:
                if True:
                    ps = psum_pool.tile([128, 1024], f32, name="ps", tag="ps")
                    si = G8C.index(g)
                    for q in range(2):
                        pb = Q8_S + pp * 1024 + q * 512
                        nc.tensor.matmul(
                            ps[:, q * 512:(q + 1) * 512],
                            Q8[:, :, si * 128:(si + 1) * 128],
                            Q8[:, :, pb:pb + 512],
                            start=True, stop=True, perf_mode=DR,
                        )
                    emit_copy(ps, g, pp)
                if pp != 3:
                    continue
                # drains: per-half 128-partition DMAs (8KB bf16 / 4KB fp8
                # per-partition descriptors), alternating HWDGE queues.
                stt, half = sthalf[g]
                if g == 4:
                    # final drain split across both queues to halve the tail
                    nc.sync.dma_start(out8[:, 20480:22528], st8c[:, 4096:6144])
                    nc.scalar.dma_start(out8[:, 22528:24576], st8c[:, 6144:8192])
                else:
                    # all mid-kernel fp8 drains issue from the idle sync
                    # engine so the scalar engine stays free for copies
                    dcol = {5: 0, 0: 4096, 1: 8192, 2: 12288, 3: 16384}[g]
                    nc.sync.dma_start(out8[:, dcol:dcol + 4096],
                                      stt[:, half * 4096:(half + 1) * 4096])

    nc.compile()
    return nc


def _pow2floor(x):
    return float(2.0 ** np.floor(np.log2(x)))


def _host_prep(z0, kernel, log_dt):
    """fp64 host math: K_discrete, powers, chunk starts; pack qin16/qin8."""
    import ml_dtypes

    BF16 = ml_dtypes.bfloat16
    FP8NP = ml_dtypes.float8_e4m3

    K = np.asarray(kernel, dtype=np.float64)
    dt = float(np.exp(np.float64(np.asarray(log_dt))))
    eye = np.eye(D, dtype=np.float64)
    A = np.linalg.solve(eye - 0.5 * dt * K, eye + 0.5 * dt * K)

    pows = [None] * (C + 1)
    pows[1] = A
    for j in range(2, C + 1):
        pows[j] = pows[j - 1] @ A

    z = np.asarray(z0, dtype=np.float64)
    s_list = [z]
    for _ in range(N_CHUNKS - 1):
        s_list.append(s_list[-1] @ pows[C])
    s_all = np.stack(s_list, axis=0)  # [32, B, D]

    # scales (powers of two)
    s8max = max(float(np.abs(s_all[4 * g: 4 * g + 4]).max()) for g in G8C)
    s8sc = _pow2floor(240.0 / (1.05 * s8max))
    p8max = max(float(np.abs(pows[j]).max()) for j in range(1, C + 1))
    p8sc = _pow2floor(240.0 / (1.05 * p8max))
    n_fp8w_chunks = 4 * len(FP8W)
    rownorm = max(
        float(np.linalg.norm(s_all[k], axis=1).max()) for k in range(n_fp8w_chunks)
    )
    colnorm = max(
        float(np.linalg.norm(pows[j], axis=0).max()) for j in range(1, C + 1)
    )
    f8s = _pow2floor(240.0 / (1.05 * rownorm * colnorm))
    eff8 = f8s / (s8sc * p8sc)

    # P16 region [128, 8192]
    p16 = np.empty((128, 8192), dtype=np.float64)
    for pp in range(4):
        for q in range(2):
            for h in range(2):
                for u2 in range(2):
                    j = 4 * pp + 1 + 2 * q + u2
                    base = pp * 2048 + q * 1024 + h * 512 + u2 * 256
                    p16[:, base:base + 256] = pows[j][h * 128:(h + 1) * 128, :]

    # P8 per-h region [2][128, 4096]
    p8h = np.empty((2, 128, 4096), dtype=np.float64)
    for h in range(2):
        for pp in range(4):
            for q in range(2):
                for u2 in range(2):
                    j = 4 * pp + 1 + 2 * q + u2
                    col = pp * 1024 + q * 512 + u2 * 256
                    p8h[h][:, col:col + 256] = (
                        pows[j][h * 128:(h + 1) * 128, :] * p8sc
                    )

    def s_block(g, cb, h):
        # [128 r, 128 (a*32+b)] = s_{4g+a}[cb+b, h*128+r]
        blk = s_all[4 * g: 4 * g + 4, cb:cb + 32, h * 128:(h + 1) * 128]
        return blk.transpose(2, 0, 1).reshape(128, 128)

    in_maps = []
    for c in range(N_CORES):
        cb = c * B_LOC
        q16 = np.empty((128, Q16_COLS), dtype=np.float64)
        for g in G16C:
            gi = SIDX[g]
            for h in range(2):
                q16[:, gi * 256 + h * 128: gi * 256 + (h + 1) * 128] = s_block(g, cb, h)
        q16[:, P16_BASE:] = p16
        q8 = np.empty((128, 2 * Q8_H), dtype=np.float64)
        for h in range(2):
            off = h * Q8_H
            for si, g in enumerate(G8C):
                q8[:, off + si * 128: off + (si + 1) * 128] = s_block(g, cb, h) * s8sc
            q8[:, off + Q8_S: off + Q8_H] = p8h[h]
        in_maps.append({
            "qin16": np.ascontiguousarray(q16).astype(BF16),
            "qin8": np.ascontiguousarray(q8).astype(FP8NP),
        })
    scales = {"fp8_scale": f8s, "eff8": eff8, "s8sc": s8sc, "p8sc": p8sc}
    return in_maps, scales


def _decode(res_c, f8s):
    """Device outputs [out16 [128,8192] bf16, out8 [128,24576] fp8]
    -> [B_LOC, T, D] f32."""
    o16 = np.asarray(res_c["out16"]).astype(np.float32)
    o8 = np.asarray(res_c["out8"]).astype(np.float32) / f8s
    out = np.empty((B_LOC, T, D), dtype=np.float32)

    def put(arr, g):  # arr [128, 4096]: [a*32+b, t_local*256+d]
        out[:, 4 * g * C:(4 * g + 4) * C, :] = (
            arr.reshape(4, 32, C, D).transpose(1, 0, 2, 3).reshape(32, 4 * C, D)
        )

    put(o16[:, 0:4096], 6)
    put(o16[:, 4096:8192], 7)
    for ri, (ga, gb) in enumerate([(5, 0), (1, 2), (3, 4)]):
        put(o8[:, ri * 8192: ri * 8192 + 4096], ga)
        put(o8[:, ri * 8192 + 4096: (ri + 1) * 8192], gb)
    return out


def kernel(**inputs):
    from concourse.bass_utils import run_bass_kernel_spmd

    z0 = inputs["z0"]
    kmat = inputs["kernel"]
    log_dt = inputs["log_dt"]
    t_in = int(np.asarray(inputs["T"]))
    assert t_in == T, f"kernel hardcoded for T={T}, got {t_in}"
    assert tuple(np.shape(z0)) == (B, D)

    in_maps, scales = _host_prep(z0, kmat, log_dt)

    key = (scales["fp8_scale"], scales["eff8"], scales["p8sc"])
    if _CACHE.get("key") != key:
        _CACHE["nc"] = _build_bass(scales["eff8"], scales["fp8_scale"],
                                   scales["p8sc"])
        _CACHE["key"] = key
    nc = _CACHE["nc"]

    res = run_bass_kernel_spmd(nc, in_maps, core_ids=list(range(N_CORES)))
    outs = [_decode(res.results[c], scales["fp8_scale"]) for c in range(N_CORES)]
    return np.concatenate(outs, axis=0)
